# revision 28
# baseline (speedup 1.0000x reference)
"""CollapseLoss kernel for Trainium2, 8-way row-sharded, fp16 datapath.

Reference computation (N=16384 rows, D=128):
    x_n   = row-normalize(feature_clusters)            # F.normalize(dim=1)
    d[i]  = dot(x_n[i+1], x_n[i])        i = 0..N-2
    out   = (d + 1/(N-1))**2

Sharding: 2048 rows per core.  Host-side sharding materializes, per core, the
SBUF image xh[128, 17*128] IN FP16: partition p holds rows 16p..16p+15 of the
shard (blocks 0..15) followed by row 16(p+1) (block 16 — the t=15 partner
row; for p=127 it is the next shard's first row: the halo).  fp16 halves the
HBM roofline and unlocks the DVE 2x_1p perf mode.

Raw (unnormalized) dots S and squared norms NE are computed in fp16; the
normalization happens algebraically in the f32 finals:
    out = (S + c*sqrt(m))^2 / m,   m = NE[t] * NE[t+1]
(sqrt only scales the tiny c term, so the loose ACT Sqrt table is ample).

Structure / engine split:
  - First input chunk is a SWDGE dma_gather prepared at t~0 on GPSIMD and
    fired by trigger_dma: the transfer starts ~800ns before the HWDGE path
    could.  Remaining chunks ride HWDGE (desc-gen pipelines under chunk 1's
    transfer).  Identity gather indices come from a GPSIMD iota.
  - Products (DVE TT) and squares (ACT Square) write an interleaved
    PRSQ[p, t, {prod,sq}, 128] buffer so ONE halving fold chain
    (TT adds at fp16 2x) + one short TensorReduce yields the interleaved
    [S, NE] per-block sums — TensorReduce has no fp16 fast path, so folding
    128->16 first is ~2x cheaper than a straight reduce.
  - Block 16's norm (the halo) goes through an ACT Square+accum.
  - The output store is a SWDGE dma_scatter_add prepared early and fired by
    trigger_dma after the finals: tail cost is ~60ns dispatch + transfer
    + sem instead of the HWDGE's 625+650 desc-gen/DGE latency.  The DRAM
    out buffer ([128, 64] padded rows, host slices [:, :16]) is zeroed by an
    early inline GPSIMD store of the memset OUTB tile (scatter ADDs).
"""

import sys
import numpy as np
from contextlib import ExitStack

try:
    import concourse  # noqa: F401
except ImportError:  # grading env without the sitecustomize path
    for _p in ("/opt/trn_rl_repo", "/root/.axon_site/_ro/trn_rl_repo"):
        if _p not in sys.path:
            sys.path.append(_p)

N_ROWS = 16384
D = 128
N_CORES = 8
R = N_ROWS // N_CORES  # 2048 rows per core
P = 128                # partitions
Q = R // P             # 16 row-blocks per partition
C_CONST = 1.0 / (N_ROWS - 1)
OUTW = 64              # padded out row width (scatter elem must be 256B)

CFG = {
    # (lo, hi, mode): input chunks in block units; block 16 = halo blk.
    # NOTE: "gather" (SWDGE dma_gather prep+trigger) starts the first
    # transfer ~800ns earlier in the model but produced flaky data/crashes
    # on the real axon cores, so loads stay on the HWDGE path.
    "chunks": ((0, 6, "hwdge"), (6, 12, "hwdge"), (12, 17, "hwdge")),
    "finals_groups": ((0, 16),),
    "fold_to": 16,
    "store_mode": "kv",   # kv | hwdge
    # sqrt(m) handling for the c*sqrt(m) term: "amgm" approximates sqrt(m)
    # by (NE_t + NE_{t+1})/2 (AM-GM, <=3% err on a term that is ~1.5% of the
    # output), shortening the finals chain; "magic" is the bitcast sqrt;
    # "act" the ACT Sqrt table.
    "finals_mode": "amgm",
    # product block ranges computed on GPSIMD (Pool) instead of DVE; Pool is
    # ~4.6x slower per element but otherwise idle mid-kernel
    # NOTE: sim says ((9,11),(14,16)) here saves ~170ns, but GPSIMD
    # tensor_tensor on fp16/strided lanes returns wrong data on real HW.
    "pool_prods": (),
    "repeat": 1,
}

SQRT_MAGIC = 0x1FBD1DF5  # bitcast(i>>1 + magic) ~= sqrt, rel err <= 4.5%

_CACHE = {}


def _plan(chunks):
    """Product ranges and paired-fold ranges implied by the chunk bounds.

    Products for blocks [a,b) read AB blocks a..b; a fold pair t needs both
    the product t and the square t (squares land with their chunk).
    """
    prod, pstart = [], 0
    for i, (lo, hi, _m) in enumerate(chunks):
        last = i == len(chunks) - 1
        pend = Q if last else min(hi - 1, Q)
        if pend > pstart:
            prod.append((pstart, pend))
            pstart = pend
    return prod


def _build_nc(cfg=None):
    import concourse.bacc as bacc
    import concourse.tile as tile
    from concourse import mybir, library_config

    cfg = dict(CFG, **(cfg or {}))
    f32 = mybir.dt.float32
    f16 = mybir.dt.float16
    i16 = mybir.dt.int16
    i32 = mybir.dt.int32
    AF = mybir.ActivationFunctionType
    ALU = mybir.AluOpType
    X = mybir.AxisListType.X
    fold_to = cfg["fold_to"]
    chunks = cfg["chunks"]
    use_kv = cfg["store_mode"] == "kv"
    any_gather = any(m == "gather" for _, _, m in chunks)

    nc = bacc.Bacc(
        "TRN2",
        target_bir_lowering=False,
        debug=False,
        enable_asserts=False,
        num_devices=N_CORES,
        num_swdge_queues=2,
    )
    xh = nc.dram_tensor("xh", [P, (Q + 1) * D], f16, kind="ExternalInput").ap()
    out = nc.dram_tensor("out", [P, OUTW], f32, kind="ExternalOutput").ap()

    prod_ranges = _plan(chunks)

    with tile.TileContext(nc) as tc:
        with ExitStack() as ctx:
            data = ctx.enter_context(tc.tile_pool(name="data", bufs=1))
            scr = ctx.enter_context(tc.tile_pool(name="scr", bufs=2))
            stat = ctx.enter_context(tc.tile_pool(name="stat", bufs=1))

            for _rep in range(cfg["repeat"]):
                # PRSQ slot 2t = prod_t (t<16), slot 2t+1 = sq_t (t<=16, so
                # slot 33 = halo square); slot 32 is never written (memset
                # once) and folds into the unused SN[:, 32].
                NS = 2 * Q + 2   # 34 slots
                AB = data.tile([P, (Q + 1) * D], f16, name=f"AB{_rep}")
                PRSQ = data.tile([P, NS * D], f16)
                F1 = data.tile([P, NS * 64], f16)
                F2 = data.tile([P, NS * 32], f16)
                F3 = data.tile([P, NS * 16], f16)
                SN = stat.tile([P, NS], f32)            # interleaved S/NE
                OUTB = stat.tile([P, OUTW], f32)

                # ---- GPSIMD stream: library, identity idxs, preps/triggers
                if any_gather or use_kv:
                    nc.gpsimd.load_library(library_config.attnmlp)
                if any_gather:
                    idxs = scr.tile([16, 8], i16, name=f"idx{_rep}")
                    # slot i of the SWDGE ring reads idxs[i%16, i//16]; we
                    # want slot i -> row i (identity).
                    nc.gpsimd.iota(idxs, pattern=[[16, 8]], base=0,
                                   channel_multiplier=1)

                # hoist the single ACT table load (Sqrt set) to t~0
                dum = scr.tile([P, 1], f32, name=f"dum{_rep}")
                one = nc.const_aps.aps[(f32, 1.0)]
                nc.scalar.activation(out=dum, in_=one[:P], func=AF.Sqrt)

                # ---- input loads
                for ci, (lo, hi, mode) in enumerate(chunks):
                    w = (hi - lo) * D
                    if mode == "gather":
                        sem = nc.alloc_semaphore(f"gat{_rep}_{ci}")
                        dst = AB[:, lo * D:hi * D].rearrange(
                            "p (s e) -> p s e", s=1)
                        nc.gpsimd.dma_gather(
                            dst, xh[:, lo * D:hi * D], idxs, P, P, w,
                            elem_step=(Q + 1) * D,
                            prepare_only=True, sem=sem)
                        nc.gpsimd.trigger_dma(count=None)
                    elif mode == "swdge":
                        # inline GPSIMD SWDGE: desc-gen starts at t~0 on the
                        # Pool engine, ~800ns before the HWDGE path can fire
                        nc.gpsimd.dma_start(out=AB[:, lo * D:hi * D],
                                            in_=xh[:, lo * D:hi * D])
                    else:
                        nc.sync.dma_start(out=AB[:, lo * D:hi * D],
                                          in_=xh[:, lo * D:hi * D])



                # ---- store prep (early; OUTB read deferred to the trigger)
                if use_kv:
                    ctxz = scr.tile([P, 1], i32, name=f"ctx{_rep}")
                    nc.gpsimd.memset(ctxz, 0)
                    ssem = nc.alloc_semaphore(f"kv{_rep}")
                    out4 = out.rearrange("p (b o q) -> b p o q", b=1, o=1)
                    in4 = OUTB[:, 0:Q].rearrange("p (o b q) -> p o b q",
                                                 o=1, b=1)
                    nc.gpsimd.kv_writeback(out4, in4, ctxz,
                                           prepare_only=True, sem=ssem,
                                           queue_num=1)

                AB3 = AB.rearrange("p (q d) -> p q d", q=Q + 1)
                PRSQt = PRSQ.rearrange("p (t x) -> p t x", t=Q + 1)  # x=2*128
                PRSQu = PRSQ.rearrange("p (u d) -> p u d", u=NS)
                V1 = F1.rearrange("p (u d) -> p u d", u=NS)
                V2 = F2.rearrange("p (u d) -> p u d", u=NS)
                V3 = F3.rearrange("p (u d) -> p u d", u=NS)

                def fold_pairs(fa, fb):
                    """Fold PRSQ pair-blocks [fa,fb) down to SN[:, 2fa:2fb]."""
                    ua, ub = 2 * fa, 2 * fb
                    nc.vector.tensor_tensor(
                        out=V1[:, ua:ub], in0=PRSQu[:, ua:ub, 0:64],
                        in1=PRSQu[:, ua:ub, 64:128], op=ALU.add)
                    cur, width = V1, 64
                    for nxt, nw in ((V2, 32), (V3, 16)):
                        if width <= fold_to:
                            break
                        nc.vector.tensor_tensor(
                            out=nxt[:, ua:ub], in0=cur[:, ua:ub, 0:nw],
                            in1=cur[:, ua:ub, nw:2 * nw], op=ALU.add)
                        cur, width = nxt, nw
                    nc.vector.tensor_reduce(SN[:, ua:ub], cur[:, ua:ub],
                                            axis=X, op=ALU.add)

                fin_groups = cfg["finals_groups"]
                fired = set()
                fold_done = 0
                sq_done = 0
                for ri, (pa, pb) in enumerate(prod_ranges):
                    lo, hi, _m = chunks[ri]
                    # squares on ACT; the last range covers the halo block
                    # whose square lands in slot 33
                    qa, qb = sq_done, min(hi, Q + 1)
                    if qb > qa:
                        nc.scalar.activation(
                            out=PRSQt[:, qa:qb, D:2 * D],
                            in_=AB3[:, qa:qb, :], func=AF.Square)
                        sq_done = qb
                    if qb == Q + 1:
                        # fill the dead prod slot 32 with the halo square too
                        # (fold lane must be finite; SN[:, 32] is unused)
                        nc.scalar.activation(
                            out=PRSQ[:, 32 * D:33 * D],
                            in_=AB3[:, Q, :], func=AF.Square)
                    # shifted products: DVE by default, Pool for the
                    # configured offload ranges
                    segs = []
                    cur = pa
                    for oa, ob in sorted(cfg["pool_prods"]):
                        oa, ob = max(oa, pa), min(ob, pb)
                        if ob <= oa:
                            continue
                        if oa > cur:
                            segs.append((cur, oa, nc.vector))
                        segs.append((oa, ob, nc.gpsimd))
                        cur = ob
                    if pb > cur:
                        segs.append((cur, pb, nc.vector))
                    for sa, sb, eng in segs:
                        eng.tensor_tensor(
                            out=PRSQt[:, sa:sb, 0:D],
                            in0=AB3[:, sa:sb, :],
                            in1=AB[:, sa * D + D:sb * D + D].rearrange(
                                "p (q d) -> p q d", q=sb - sa),
                            op=ALU.mult)
                    # fold every pair with both lanes ready (pair 16 has
                    # only the sq lane; its prod slot 32 is the memset slot)
                    fb = Q + 1 if sq_done == Q + 1 else min(pb, sq_done)
                    if fb > fold_done:
                        fold_pairs(fold_done, fb)
                        fold_done = fb

                    for gi, (ga, gb) in enumerate(fin_groups):
                        if gi not in fired and fold_done >= gb + 1:
                            fired.add(gi)
                            _emit_finals(nc, stat, mybir, SN, OUTB,
                                         ga, gb, gi, cfg)

                if use_kv:
                    nc.gpsimd.trigger_dma(count=None, queue_num=1)
                else:
                    nc.sync.dma_start(out=out[:, 0:Q], in_=OUTB[:, 0:Q])

    nc.compile()
    _fix_prep_sems(nc, mybir)
    return nc


def _fix_prep_sems(nc, mybir):
    """Point each SWDGE prep's baked DMA-completion sem at its Tile lane sem.

    Tile assigns every Pool DMA inst (including gen_mode==1 preps) a DMASW
    lane and makes downstream waiters wait on that lane's semaphore, but the
    prepare_only API bakes the caller-provided sem into the descriptor
    (on_update[0]) and Tile never rewrites it — so the lane sem would never
    fire.  Rewrite on_update[0] to the lane sem the waiters expect.
    """
    from concourse.tile_sem_assignment import PROC_NAME_TO_IDX

    idx_to_lane = {v: k for k, v in PROC_NAME_TO_IDX.items()}
    by_lane = {}
    for sid, names in nc.m.ant_sem_names.items():
        for nm in names:
            by_lane.setdefault(nm.split("_")[0], (int(sid), nm))
    for f in nc.m.functions:
        for blk in f.blocks:
            for inst in blk.instructions:
                if getattr(inst, "gen_mode", 0) != 1:
                    continue
                lane = idx_to_lane.get(inst.bass_scheduled_proc)
                if lane is None or lane not in by_lane:
                    continue
                sid, nm = by_lane[lane]
                u0 = inst.sync_info.on_update[0]
                inst.sync_info.on_update[0] = mybir.SyncUpdate(
                    sync_type=u0.sync_type, id=sid, ant_name=nm,
                    update_mode=u0.update_mode, update_value=16)


def _emit_finals(nc, stat, mybir, SN, OUTB, ga, gb, gi, cfg):
    """OUTB[:, ga:gb] = (S + c*sqrt(m))^2 / m for block range [ga, gb)."""
    ALU = mybir.AluOpType
    f32 = mybir.dt.float32
    i32 = mybir.dt.int32
    AF = mybir.ActivationFunctionType
    w_ = gb - ga
    SNt = SN.rearrange("p (t s) -> p t s", s=2)
    Sv = SNt[:, ga:gb, 0:1]
    NEv = SNt[:, :, 1:2]
    m = stat.tile([P, w_], f32, name=f"m{gi}")
    m3 = m.rearrange("p (t s) -> p t s", s=1)
    nc.vector.tensor_tensor(out=m3, in0=NEv[:, ga:gb],
                            in1=NEv[:, ga + 1:gb + 1], op=ALU.mult)
    w = stat.tile([P, w_], f32, name=f"w{gi}")
    nc.vector.reciprocal(w, m)   # off the sqrt chain; joins at the end
    u = stat.tile([P, w_], f32, name=f"u{gi}")
    u3 = u.rearrange("p (t s) -> p t s", s=1)
    mode = cfg["finals_mode"]
    if mode == "amgm":
        # sqrt(NE_t*NE_t1) ~= (NE_t+NE_t1)/2; u = S + (c/2)*(NE_t+NE_t1)
        ns_ = stat.tile([P, w_], f32, name=f"ns{gi}")
        ns3 = ns_.rearrange("p (t s) -> p t s", s=1)
        nc.vector.tensor_tensor(out=ns3, in0=NEv[:, ga:gb],
                                in1=NEv[:, ga + 1:gb + 1], op=ALU.add)
        nc.vector.scalar_tensor_tensor(out=u3, in0=ns3, scalar=C_CONST / 2,
                                       in1=Sv, op0=ALU.mult, op1=ALU.add)
    else:
        s0 = stat.tile([P, w_], f32, name=f"s0{gi}")
        if mode == "act":
            nc.scalar.activation(out=s0, in_=m, func=AF.Sqrt)
        else:
            sh = stat.tile([P, w_], f32, name=f"sh{gi}")
            nc.vector.tensor_scalar(sh.bitcast(i32), m.bitcast(i32), 1, None,
                                    ALU.logical_shift_right)
            nc.vector.tensor_scalar(s0.bitcast(i32), sh.bitcast(i32),
                                    SQRT_MAGIC, None, ALU.add)
        nc.vector.scalar_tensor_tensor(out=u3,
                                       in0=s0.rearrange("p (t s) -> p t s",
                                                        s=1),
                                       scalar=C_CONST, in1=Sv,
                                       op0=ALU.mult, op1=ALU.add)
    v = stat.tile([P, w_], f32, name=f"v{gi}")
    nc.vector.tensor_tensor(out=v, in0=u, in1=u, op=ALU.mult)
    nc.vector.tensor_tensor(out=OUTB[:, ga:gb], in0=v, in1=w, op=ALU.mult)


def _get_nc():
    if "nc" not in _CACHE:
        _CACHE["nc"] = _build_nc()
    return _CACHE["nc"]


def make_in_maps(x: np.ndarray) -> list[dict[str, np.ndarray]]:
    """Host-side sharding: build each core's SBUF image xh[128, 2176] fp16."""
    x = np.asarray(x, dtype=np.float32).astype(np.float16)
    # pad one row (the out-of-range halo of the last core) with ones
    xp = np.concatenate([x, np.ones((1, D), dtype=np.float16)], axis=0)
    in_maps = []
    for c in range(N_CORES):
        sh = xp[c * R:c * R + R].reshape(P, Q * D)        # blocks 0..15
        halo = xp[c * R + 16 * np.arange(1, P + 1)]       # block 16
        xh = np.concatenate([sh, halo.reshape(P, D)], axis=1)
        in_maps.append({"xh": np.ascontiguousarray(xh)})
    return in_maps


def kernel(feature_clusters: np.ndarray) -> np.ndarray:
    from concourse.bass_utils import run_bass_kernel_spmd

    nc = _get_nc()
    in_maps = make_in_maps(feature_clusters)
    res = run_bass_kernel_spmd(nc, in_maps, list(range(N_CORES))).results
    full = np.concatenate(
        [res[c]["out"][:, :Q].reshape(R) for c in range(N_CORES)])
    return full[:N_ROWS - 1].astype(np.float32)
